# revision 10
# baseline (speedup 1.0000x reference)
"""Trainium2 Bass kernel for the minGRU encoder (nn_Encoder_65635690218112).

Strategy
--------
- Data-parallel over batch: 16 batches -> 8 cores x 2 batches.
- Feature-major (h^T) layout [D, T] end-to-end; transpose-free.
- Mixed-precision matmuls (tolerance is 2e-2; this lands ~6e-3):
    * gate (z) path: fp8-e4m3 DoubleRow matmuls (2 contraction rows per
      partition -> 4 matmuls of K=256 instead of 8 of K=128). Weights are
      quantized per output column with power-of-2 scales; h is stored as
      h*2^k_i in fp8 pair tiles. The sigmoid undoes both scales via a
      per-partition scale AP.
    * candidate (c) path, pre, post: bf16 (enables fast weight load).
- The scan (h_t = a_t*h_{t-1} + b_t) runs on the vector engine via
  tensor_tensor_scan and writes h directly as bf16 (fp32 internal state);
  a separate vector op makes the scaled fp8 copy for the next layer's
  z-path matmuls.
- Time axis split into G=2 groups of 2048 so h + weights fit SBUF;
  per-layer carry columns bridge the groups.
"""

import numpy as np
import ml_dtypes

import concourse.bass as bass
import concourse.mybir as mybir
import concourse.tile as tile

# ---------------------------------------------------------------------------
# Workaround: this walrus build accepts at most ONE sem wait per instruction
# ("Too many sync wait commands"). After Tile assigns waits, split any
# instruction carrying more by inserting same-engine NoOps ahead of it.
# ---------------------------------------------------------------------------
from concourse.vector_clock import ScopedClock

_MAX_WAITS = 1
_noop_ctr = [0]


def _split_waits_in_block(bb):
    new_list = []
    for inst in bb.instructions:
        si = getattr(inst, "sync_info", None)
        if si is not None and si.on_wait and len(si.on_wait) > _MAX_WAITS:
            waits = list(si.on_wait)
            keep = waits[-_MAX_WAITS:]
            extra = waits[:-_MAX_WAITS]
            for i in range(0, len(extra), _MAX_WAITS):
                _noop_ctr[0] += 1
                nop = mybir.InstNoOp(
                    name=f"I-waitsplit-{_noop_ctr[0]}",
                    engine=inst.engine,
                    bass_nofuse=True,
                    sync_info=mybir.SyncInfo(
                        on_wait=extra[i : i + _MAX_WAITS], on_update=[]
                    ),
                )
                new_list.append(nop)
            inst.sync_info = mybir.SyncInfo(on_wait=keep, on_update=si.on_update)
        new_list.append(inst)
    bb.instructions[:] = new_list


def _patched_drain_and_barrier(self, tick_clock, wait_clock):
    nc = self.nc
    drain_inst = nc.sync.drain()
    wait_clock.add_sem_waits(
        drain_inst.ins, ScopedClock({None: tick_clock.global_clock})
    )
    for bb in nc.main_func.blocks:
        _split_waits_in_block(bb)
    nc.all_engine_barrier()
    assert self.sems is not None
    popped = nc._tile_sem_poison_stack.pop()
    assert popped is self._sem_poison
    nc.clear_and_free_semaphores(list(self.sems.allocated().values()))
    nc.all_engine_barrier()


tile.TileContext._drain_and_barrier = _patched_drain_and_barrier

# ---------------------------------------------------------------------------

f32 = mybir.dt.float32
bf16 = mybir.dt.bfloat16
fp8 = mybir.dt.float8e4
AF = mybir.ActivationFunctionType
ALU = mybir.AluOpType
DR = mybir.MatmulPerfMode.DoubleRow

N_CORES = 8
B_FULL = 16
C_IN = 80
C_OUT = 194
D = 1024
NJ = D // 128  # 8 feature blocks of 128
NQ = NJ // 2  # 4 fp8 pair blocks

# per-layer-input scale exponents for the fp8 h store (h8 = h * 2^k).
# |h| maxes decay ~0.38 -> 0.02 over the layers; these keep max|h8| ~ 10-90
# with >2x headroom to the fp8e4 ceiling (240).
K_TABLE = (6, 7, 8, 9, 10, 11, 12)


def build_program_v2(L=6, T=4096, G=2, S=512, BS=2, REP=1, z_dr=True, lw2=False):
    """Mixed-precision SPMD per-core program. Returns nc.

    lw2=True issues each weight block's matmuls for 2 chunks back-to-back
    (amortizes LDWEIGHTS when the hardware can keep weights stationary).
    """
    Tg = T // G
    NCH = Tg // S
    nc = bass.Bass()

    x_d = nc.declare_dram_parameter("x", [BS, C_IN, T], bf16, isOutput=False)
    wpre_d = nc.declare_dram_parameter("wpre", [C_IN, D], bf16, isOutput=False)
    # pb: [bpre | bpre * 2^K0] as [128, 2*NJ]
    pb_d = nc.declare_dram_parameter("pb", [128, 2 * NJ], f32, isOutput=False)
    if z_dr:
        wz_d = nc.declare_dram_parameter(
            "wz", [L, NJ, 128, NQ, 2, 128], fp8, isOutput=False
        )
    else:
        wz_d = nc.declare_dram_parameter("wz", [L, NJ, 128, D], bf16, isOutput=False)
    wc_d = nc.declare_dram_parameter("wc", [L, NJ, 128, D], bf16, isOutput=False)
    # sc: per-(layer, feature) scalars [sz | -sz | bz | -bz | bc] as [L,128,5*NJ]
    sc_d = nc.declare_dram_parameter("sc", [L, 128, 5 * NJ], f32, isOutput=False)
    wpost_d = nc.declare_dram_parameter(
        "wpost", [128, NJ * C_OUT], bf16, isOutput=False
    )
    bpost_d = nc.declare_dram_parameter("bpost", [128, 2], f32, isOutput=False)
    out_d = nc.declare_dram_parameter("out", [BS, C_OUT, T], f32, isOutput=True)

    with tile.TileContext(nc) as tc:
        with (
            tc.tile_pool(name="const", bufs=1) as cpool,
            tc.tile_pool(name="h", bufs=1) as hpool,
            tc.tile_pool(name="w", bufs=1) as wpool,
            tc.tile_pool(name="bias", bufs=1) as bpool,
            tc.tile_pool(name="scr", bufs=1) as spool,
            tc.tile_pool(name="ps", bufs=1, space="PSUM") as pspool,
        ):
            # ---- constants loaded once ----
            wpre_sb = cpool.tile([C_IN, D], bf16, tag="wpre")
            nc.sync.dma_start(wpre_sb[:], wpre_d[:])
            pb_sb = cpool.tile([128, 2 * NJ], f32, tag="pb")
            nc.sync.dma_start(pb_sb[:], pb_d[:])
            wpost_sb = cpool.tile([128, NJ * C_OUT], bf16, tag="wpost")
            nc.sync.dma_start(wpost_sb[:], wpost_d[:])
            bpost_sb = cpool.tile([128, 2], f32, tag="bpost")
            nc.sync.dma_start(bpost_sb[:], bpost_d[:])
            carry_sb = cpool.tile([128, L * NJ], f32, tag="carry")

            # persistent h tiles per (feature-block, chunk):
            #   hb: true-scale bf16 (c-path + post + scan carries)
            #   h8: 2^k-scaled fp8 pair tiles (z-path DoubleRow rhs)
            hb = [
                [hpool.tile([128, S], bf16, tag=f"hb{j}_{c}", name=f"hb{j}_{c}")
                 for c in range(NCH)]
                for j in range(NJ)
            ]
            h8 = [
                [hpool.tile([128, 2, S], fp8, tag=f"h8{q}_{c}", name=f"h8{q}_{c}")
                 for c in range(NCH)]
                for q in range(NQ)
            ]

            def h8_slice(j, c):
                ap = h8[j // 2][c][:, j % 2]
                assert tuple(ap.shape) == (128, S), ap.shape
                return ap

            for _rep in range(REP):
              for b in range(BS):
                for g in range(G):
                    t0 = g * Tg
                    # ---- input slab for this (batch, group) ----
                    x_sb = spool.tile([C_IN, Tg], bf16, tag="x", name="x_sb")
                    nc.sync.dma_start(x_sb[:], x_d[b][:, t0 : t0 + Tg])

                    # ---- pre-projection: h = x^T W_pre + b_pre ----
                    for c in range(NCH):
                        for j in range(NJ):
                            ps = pspool.tile(
                                [128, S], f32,
                                tag=("psz" if j % 2 == 0 else "psc"), bufs=4,
                                name="ps_pre",
                            )
                            nc.tensor.matmul(
                                ps[:],
                                wpre_sb[:, j * 128 : (j + 1) * 128],
                                x_sb[:, c * S : (c + 1) * S],
                                start=True,
                                stop=True,
                            )
                            nc.scalar.activation(
                                hb[j][c][:], ps[:], AF.Identity,
                                bias=pb_sb[:, j : j + 1], scale=1.0,
                            )
                            if z_dr:
                                nc.vector.tensor_scalar(
                                    h8_slice(j, c), hb[j][c][:],
                                    float(2.0 ** K_TABLE[0]), None,
                                    op0=ALU.mult,
                                )

                    # ---- the L minGRU layers ----
                    for i in range(L):
                        if z_dr:
                            wzt = []
                            for j in range(NJ):
                                wt = wpool.tile(
                                    [128, NQ, 2, 128], fp8, tag=f"wz{j}",
                                    name=f"wz{j}",
                                )
                                nc.sync.dma_start(wt[:], wz_d[i, j])
                                wzt.append(wt)
                        else:
                            wzt = []
                            for j in range(NJ):
                                wt = wpool.tile(
                                    [128, D], bf16, tag=f"wz{j}", name=f"wz{j}"
                                )
                                nc.sync.dma_start(wt[:], wz_d[i, j])
                                wzt.append(wt)
                        wct = []
                        for j in range(NJ):
                            wt = wpool.tile(
                                [128, D], bf16, tag=f"wc{j}", name=f"wc{j}"
                            )
                            nc.sync.dma_start(wt[:], wc_d[i, j])
                            wct.append(wt)
                        sct = bpool.tile(
                            [128, 5 * NJ], f32, tag="sc", bufs=2, name="sc"
                        )
                        nc.sync.dma_start(sct[:], sc_d[i])

                        CB = 2 if lw2 else 1  # chunks sharing one weight load
                        for cp in range(0, NCH, CB):
                            cs = list(range(cp, cp + CB))
                            a_ts = {}
                            b_ts = {}
                            for j in range(NJ):
                                psz = {c: pspool.tile(
                                    [128, S], f32, tag="psz", bufs=4, name="psz"
                                ) for c in cs}
                                psc = {c: pspool.tile(
                                    [128, S], f32, tag="psc", bufs=4, name="psc"
                                ) for c in cs}
                                if z_dr:
                                    for q in range(NQ):
                                        wap = wzt[j][:, q]
                                        assert tuple(wap.shape) == (128, 2, 128), wap.shape
                                        for c in cs:
                                            nc.tensor.matmul(
                                                psz[c][:],
                                                wap,
                                                h8[q][c][:],
                                                start=(q == 0),
                                                stop=(q == NQ - 1),
                                                perf_mode=DR,
                                            )
                                else:
                                    for kb in range(NJ):
                                        for c in cs:
                                            nc.tensor.matmul(
                                                psz[c][:],
                                                wzt[j][:, kb * 128 : (kb + 1) * 128],
                                                hb[kb][c][:],
                                                start=(kb == 0),
                                                stop=(kb == NJ - 1),
                                            )
                                for kb in range(NJ):
                                    for c in cs:
                                        nc.tensor.matmul(
                                            psc[c][:],
                                            wct[j][:, kb * 128 : (kb + 1) * 128],
                                            hb[kb][c][:],
                                            start=(kb == 0),
                                            stop=(kb == NJ - 1),
                                        )
                                for c in cs:
                                    z_t = spool.tile(
                                        [128, S], f32, tag="z", bufs=4, name="z_t"
                                    )
                                    a_t = spool.tile(
                                        [128, S], f32, tag="a", bufs=9 + 8 * (CB - 1),
                                        name="a_t",
                                    )
                                    b_t = spool.tile(
                                        [128, S], f32, tag="bb", bufs=9 + 8 * (CB - 1),
                                        name="b_t",
                                    )
                                    # z = sigmoid(psz * sz + bz)
                                    nc.scalar.activation(
                                        z_t[:], psz[c][:], AF.Sigmoid,
                                        bias=sct[:, 2 * NJ + j : 2 * NJ + j + 1],
                                        scale=sct[:, j : j + 1],
                                    )
                                    # a = 1 - z = sigmoid(-psz * sz - bz)
                                    nc.scalar.activation(
                                        a_t[:], psz[c][:], AF.Sigmoid,
                                        bias=sct[:, 3 * NJ + j : 3 * NJ + j + 1],
                                        scale=sct[:, NJ + j : NJ + j + 1],
                                    )
                                    # b = (psc + bc) * z
                                    nc.vector.scalar_tensor_tensor(
                                        b_t[:], psc[c][:],
                                        sct[:, 4 * NJ + j : 4 * NJ + j + 1], z_t[:],
                                        op0=ALU.add, op1=ALU.mult,
                                    )
                                    a_ts[j, c] = a_t
                                    b_ts[j, c] = b_t
                            # scans run after ALL matmuls of these chunks
                            for c in cs:
                                for j in range(NJ):
                                    if g == 0 and c == 0:
                                        init = 0.0
                                    elif c == 0:
                                        init = carry_sb[:, i * NJ + j : i * NJ + j + 1]
                                    else:
                                        init = hb[j][c - 1][:, S - 1 : S]
                                    nc.vector.tensor_tensor_scan(
                                        hb[j][c][:], a_ts[j, c][:], b_ts[j, c][:],
                                        init, op0=ALU.mult, op1=ALU.add,
                                    )
                                    if z_dr and i < L - 1:
                                        nc.vector.tensor_scalar(
                                            h8_slice(j, c), hb[j][c][:],
                                            float(2.0 ** K_TABLE[i + 1]), None,
                                            op0=ALU.mult,
                                        )
                        if g == 0:
                            for j in range(NJ):
                                nc.vector.tensor_copy(
                                    carry_sb[:, i * NJ + j : i * NJ + j + 1],
                                    hb[j][NCH - 1][:, S - 1 : S],
                                )

                    # ---- post-projection: out = h^T W_post + b_post ----
                    for c in range(NCH):
                        for p, (p0, pw) in enumerate(((0, 128), (128, C_OUT - 128))):
                            ps_o = pspool.tile(
                                [128, S], f32,
                                tag=("psz" if p == 0 else "psc"), bufs=4,
                                name="ps_o",
                            )
                            for kb in range(NJ):
                                nc.tensor.matmul(
                                    ps_o[:pw, :],
                                    wpost_sb[
                                        :, kb * C_OUT + p0 : kb * C_OUT + p0 + pw
                                    ],
                                    hb[kb][c][:],
                                    start=(kb == 0),
                                    stop=(kb == NJ - 1),
                                )
                            o_t = spool.tile([128, S], f32, tag="o", bufs=4, name="o_t")
                            if p == 0:
                                nc.scalar.activation(
                                    o_t[:pw, :], ps_o[:pw, :], AF.Identity,
                                    bias=bpost_sb[:pw, p : p + 1], scale=1.0,
                                )
                            else:
                                nc.vector.tensor_scalar(
                                    o_t[:pw, :], ps_o[:pw, :],
                                    bpost_sb[:pw, p : p + 1], None, op0=ALU.add,
                                )
                            nc.sync.dma_start(
                                out_d[b][p0 : p0 + pw, t0 + c * S : t0 + (c + 1) * S],
                                o_t[:pw, :],
                            )
    return nc


def pack_inputs_v2(x, w_pre, b_pre, w_layers, b_layers, w_post, b_post,
                   L=6, z_dr=True):
    """Host-side packing into DMA-friendly layouts."""
    nbf = ml_dtypes.bfloat16
    nf8 = ml_dtypes.float8_e4m3

    x = np.ascontiguousarray(np.asarray(x, dtype=np.float32).astype(nbf))
    wpre = np.ascontiguousarray(np.asarray(w_pre, dtype=np.float32).astype(nbf))
    bpre = np.asarray(b_pre, dtype=np.float32).reshape(NJ, 128).T  # [128, NJ]
    pb = np.concatenate([bpre, bpre * 2.0 ** K_TABLE[0]], axis=1)
    pb = np.ascontiguousarray(pb.astype(np.float32))

    wl = np.asarray(w_layers, dtype=np.float32)
    Wz = wl[:, :, :D]  # [L, K, M]
    Wc = wl[:, :, D:]

    # c-path weights: wc[i, j, p, kb*128+m] = Wc[i, kb*128+p, j*128+m]
    wc = (
        Wc.reshape(L, NJ, 128, NJ, 128)
        .transpose(0, 3, 2, 1, 4)
        .reshape(L, NJ, 128, D)
    )
    wc = np.ascontiguousarray(wc.astype(nbf))

    bl = np.asarray(b_layers, dtype=np.float32)
    bz = bl[:, :D].reshape(L, NJ, 128).transpose(0, 2, 1)  # [L, 128, NJ]
    bc = bl[:, D:].reshape(L, NJ, 128).transpose(0, 2, 1)

    if z_dr:
        colmax = np.abs(Wz).max(axis=1, keepdims=True)  # [L, 1, M]
        colscale = 2.0 ** np.floor(np.log2(128.0 / np.maximum(colmax, 1e-30)))
        Wzs = Wz * colscale
        # wz8[i, j, p, q, i2, m] = Wzs[i, (2q+i2)*128+p, j*128+m]
        wz8 = (
            Wzs.reshape(L, NQ, 2, 128, NJ, 128)
            .transpose(0, 4, 3, 1, 2, 5)
        )
        wz8 = np.ascontiguousarray(np.clip(wz8, -240.0, 240.0).astype(nf8))
        # sigmoid scale undoes colscale and the h8 scale 2^k_i
        ks = np.array([2.0 ** K_TABLE[i] for i in range(L)], np.float32)
        sz = 1.0 / (colscale[:, 0, :] * ks[:, None])  # [L, M]
        sz = sz.reshape(L, NJ, 128).transpose(0, 2, 1)  # [L, 128, NJ]
    else:
        wz8 = (
            Wz.reshape(L, NJ, 128, NJ, 128)
            .transpose(0, 3, 2, 1, 4)
            .reshape(L, NJ, 128, D)
        )
        wz8 = np.ascontiguousarray(wz8.astype(nbf))
        sz = np.ones((L, 128, NJ), np.float32)

    sc = np.concatenate([sz, -sz, bz, -bz, bc], axis=2)  # [L, 128, 5*NJ]
    sc = np.ascontiguousarray(sc.astype(np.float32))

    # wpost[kp, kb*C_OUT + c] = w_post[kb*128+kp, c]
    wpost = (
        np.asarray(w_post, dtype=np.float32)
        .reshape(NJ, 128, C_OUT)
        .transpose(1, 0, 2)
        .reshape(128, NJ * C_OUT)
    )
    wpost = np.ascontiguousarray(wpost.astype(nbf))
    bpost = np.zeros((128, 2), dtype=np.float32)
    bpost[:, 0] = np.asarray(b_post[:128], dtype=np.float32)
    bpost[: C_OUT - 128, 1] = np.asarray(b_post[128:], dtype=np.float32)
    return x, wpre, pb, wz8, wc, sc, wpost, bpost


_program_cache = {}


def _get_program(key):
    if key not in _program_cache:
        L, T, G, S, BS, REP, z_dr = key[:7]
        lw2 = key[7] if len(key) > 7 else False
        _program_cache[key] = build_program_v2(
            L=L, T=T, G=G, S=S, BS=BS, REP=REP, z_dr=z_dr, lw2=lw2
        )
    return _program_cache[key]


def make_in_maps(inputs, L=6, z_dr=True):
    x, wpre, pb, wz8, wc, sc, wpost, bpost = pack_inputs_v2(
        inputs["x"], inputs["w_pre"], inputs["b_pre"], inputs["w_layers"],
        inputs["b_layers"], inputs["w_post"], inputs["b_post"], L=L, z_dr=z_dr,
    )
    B = x.shape[0]
    BS = B // N_CORES
    shared = {"wpre": wpre, "pb": pb, "wz": wz8, "wc": wc, "sc": sc,
              "wpost": wpost, "bpost": bpost}
    in_maps = [
        {"x": np.ascontiguousarray(x[c * BS : (c + 1) * BS]), **shared}
        for c in range(N_CORES)
    ]
    return in_maps, BS


def run(inputs, L=6, T=4096, G=2, S=512, REP=1, z_dr=True, lw2=False,
        trace=False):
    """Run the SPMD kernel on the full inputs; returns (out, bass_results)."""
    from concourse.bass_utils import run_bass_kernel_spmd

    in_maps, BS = make_in_maps(inputs, L=L, z_dr=z_dr)
    nc = _get_program((L, T, G, S, BS, REP, z_dr, lw2))
    res = run_bass_kernel_spmd(
        nc, in_maps, list(range(N_CORES)), trace=trace
    )
    out = np.concatenate([res.results[c]["out"] for c in range(N_CORES)], axis=0)
    return out, res


def kernel(**inputs):
    out, _ = run(inputs)
    return out
